# revision 8
# baseline (speedup 1.0000x reference)
"""Multi-head causal attention (B=4, S=2048, D=1024, H=16) on 8 Trainium2 cores.

Strategy: tensor-parallel over heads (2 heads/core).
 - Host feeds each core xT = x^T [D, B*S] (fp32r + bf16 copies) plus that
   core's slice of w_in columns (q cols pre-scaled by 1/sqrt(dh)), and full
   w_out.
 - Phase 1: k,v chains in fp32r (output precision), q chain in bf16;
   qT/kT resident in SBUF; vT staged + PE-transposed to v-natural bf16 tiles
   with a ones column appended (flash-attention sum trick).
 - Phase 2 (bf16 matmuls): per (batch, head): scoresT[k,q] pairs of k-tiles
   into one 2-bank PSUM tile, one Exp per pair (ACT), causal mask multiply,
   ctxT[dh+1, q] accumulated on PE with v_aug stationary; row dh = sum(exp).
   Software-pipelined (next pair's scores before this pair's AV matmuls).
   Normalize via partition_broadcast + reciprocal_approx_fast.
 - Four AllToAlls (one per batch) reshard ctxT from head-split to row-split;
   all overlap attention/out-proj compute.
 - Phase 3: out rows-slice = ctxT_full^T @ w_out + b_out (fp32r).
Outputs per core: kT/vT head slices and out rows-slices; host reassembles.
"""

import numpy as np
import ml_dtypes
from contextlib import ExitStack

NCORES = 8
DH = 64
H = 16
HPC = H // NCORES          # heads per core = 2
D = H * DH                 # 1024
NKD = D // 128             # 8 contraction tiles over D
RC = 512                   # phase-1 row chunk
QC = 512                   # phase-2 query chunk

_CACHE = {}


def _build(B=4, S=2048):
    import concourse.tile as tile
    from concourse import bacc, mybir
    from concourse.masks import make_identity

    R = B * S
    W = S // NCORES        # per-batch A2A shard width (rows)
    NQC = S // QC
    f32 = mybir.dt.float32
    f32r = mybir.dt.float32r
    bf16 = mybir.dt.bfloat16
    EXP = mybir.ActivationFunctionType.Exp

    nc = bacc.Bacc("TRN2", target_bir_lowering=False, debug=False,
                   num_devices=NCORES)

    xT = nc.dram_tensor("xT", [D, R], f32r, kind="ExternalInput").ap()
    x16 = nc.dram_tensor("x16", [D, R], bf16, kind="ExternalInput").ap()
    w_kv = nc.dram_tensor("w_kv", [D, 2 * HPC * DH], f32r,
                          kind="ExternalInput").ap()
    w_q16 = nc.dram_tensor("w_q16", [D, HPC * DH], bf16,
                           kind="ExternalInput").ap()
    b_qkv = nc.dram_tensor("b_qkv", [1, 3 * HPC * DH], bf16,
                           kind="ExternalInput").ap()
    w_out = nc.dram_tensor("w_out", [D, D], f32r, kind="ExternalInput").ap()
    b_out = nc.dram_tensor("b_out", [1, D], bf16, kind="ExternalInput").ap()
    tri = nc.dram_tensor("tri", [128, 896], bf16, kind="ExternalInput").ap()
    cst = nc.dram_tensor("cst", [128, 512], bf16, kind="ExternalInput").ap()

    kT_out = nc.dram_tensor("kT_out", [HPC * DH, R], f32,
                            kind="ExternalOutput").ap()
    vT_out = nc.dram_tensor("vT_out", [HPC * DH, R], f32,
                            kind="ExternalOutput").ap()
    o_out = nc.dram_tensor("o_out", [B, W, D], f32,
                           kind="ExternalOutput").ap()

    # per-batch A2A buffers
    a2a_in = [nc.dram_tensor(f"a2a_in{b}", [NCORES * 128, W], f32r)
              for b in range(B)]
    a2a_out = [nc.dram_tensor(f"a2a_out{b}", [NCORES * 128, W], f32r)
               for b in range(B)]
    RG = [list(range(NCORES))]

    with tile.TileContext(nc) as tc, ExitStack() as ctx:
        persist = ctx.enter_context(tc.tile_pool(name="persist", bufs=1))
        qT = persist.tile([128, R], bf16)
        kT = persist.tile([128, R], f32)
        kT16 = persist.tile([128, R], bf16)
        vA = persist.tile([128, R // 128, HPC, DH + 1], bf16)
        triE = persist.tile([128, 896], bf16)
        ident = persist.tile([128, 128], f32)
        ones_sb = persist.tile([128, 512], bf16)
        bout_sb = persist.tile([1, D], bf16)

        make_identity(nc, ident)
        nc.sync.dma_start(out=ones_sb, in_=cst)
        nc.sync.dma_start(out=triE, in_=tri)
        nc.sync.dma_start(out=bout_sb, in_=b_out)
        # ones column of vA via one strided copy (the v copies in phase 1
        # fill the rest).
        nc.vector.tensor_copy(
            out=vA[:, :, :, DH:DH + 1],
            in_=ones_sb[:, 0:R // 128 * HPC].rearrange(
                "p (a b c) -> p a b c", b=HPC, c=1))

        # ---------------- Phase 1: qkv projection ----------------
        with tc.tile_pool(name="p1", bufs=3) as p1, \
             tc.tile_pool(name="p1w", bufs=1) as p1w, \
             tc.tile_pool(name="ps1", bufs=2, space="PSUM") as ps1:
            wkv = p1w.tile([128, NKD, 2 * HPC * DH], f32r)
            nc.sync.dma_start(out=wkv,
                              in_=w_kv.rearrange("(kt p) c -> p kt c", p=128))
            wq16 = p1w.tile([128, NKD, HPC * DH], bf16)
            nc.sync.dma_start(out=wq16,
                              in_=w_q16.rearrange("(kt p) c -> p kt c", p=128))
            bq = p1w.tile([1, 3 * HPC * DH], bf16)
            nc.sync.dma_start(out=bq, in_=b_qkv)

            for rc in range(R // RC):
                r0 = rc * RC
                xt = p1.tile([128, NKD, RC], f32r, tag="xt")
                nc.sync.dma_start(
                    out=xt,
                    in_=xT[:, r0:r0 + RC].rearrange("(kt p) r -> p kt r", p=128))
                xt16 = p1.tile([128, NKD, RC], bf16, tag="xt16")
                nc.sync.dma_start(
                    out=xt16,
                    in_=x16[:, r0:r0 + RC].rearrange("(kt p) r -> p kt r",
                                                     p=128))
                ps_q = ps1.tile([128, RC], f32, tag="psq")
                ps_k = ps1.tile([128, RC], f32, tag="psk")
                ps_v = ps1.tile([128, RC], f32, tag="psv")
                # q chain in bf16
                nc.tensor.matmul(out=ps_q[:, :], lhsT=bq[0:1, 0:128],
                                 rhs=ones_sb[0:1, 0:RC], start=True, stop=False)
                for kt in range(NKD):
                    nc.tensor.matmul(out=ps_q[:, :], lhsT=wq16[:, kt, :],
                                     rhs=xt16[:, kt, :],
                                     start=False, stop=(kt == NKD - 1))
                # k, v chains in fp32r
                for ps_t, c0 in ((ps_k, 0), (ps_v, 128)):
                    nc.tensor.matmul(out=ps_t[:, :],
                                     lhsT=bq[0:1, 128 + c0:256 + c0],
                                     rhs=ones_sb[0:1, 0:RC],
                                     start=True, stop=False)
                    for kt in range(NKD):
                        nc.tensor.matmul(out=ps_t[:, :],
                                         lhsT=wkv[:, kt, c0:c0 + 128],
                                         rhs=xt[:, kt, :],
                                         start=False, stop=(kt == NKD - 1))
                nc.vector.tensor_copy(out=qT[:, r0:r0 + RC], in_=ps_q[:, :])
                nc.vector.tensor_copy(out=kT[:, r0:r0 + RC], in_=ps_k[:, :])
                nc.vector.tensor_copy(out=kT16[:, r0:r0 + RC], in_=ps_k[:, :])
                nc.gpsimd.dma_start(out=kT_out[:, r0:r0 + RC],
                                    in_=kT[:, r0:r0 + RC])
                vt_sb = p1.tile([128, RC], f32, tag="vtsb")
                nc.vector.tensor_copy(out=vt_sb[:, :], in_=ps_v[:, :])
                nc.gpsimd.dma_start(out=vT_out[:, r0:r0 + RC],
                                    in_=vt_sb[:, :])
                for t4 in range(RC // 128):
                    ps_vt = ps1.tile([128, 128], f32, tag="psvt")
                    nc.tensor.transpose(ps_vt[:, :],
                                        vt_sb[:, t4 * 128:(t4 + 1) * 128],
                                        ident)
                    rt = rc * (RC // 128) + t4
                    for hh in range(HPC):
                        nc.vector.tensor_copy(
                            out=vA[:, rt, hh, 0:DH],
                            in_=ps_vt[:, hh * DH:(hh + 1) * DH])

        # ---- Phase 2 (attention, heads packed as concurrent row-tiles) ----
        # ---- interleaved with per-batch A2A + Phase 3 out-proj ----
        with tc.tile_pool(name="p2", bufs=4) as p2, \
             tc.tile_pool(name="p2s", bufs=2) as p2s, \
             tc.tile_pool(name="p3", bufs=2) as p3, \
             tc.tile_pool(name="p3w", bufs=1) as p3w, \
             tc.tile_pool(name="p3o", bufs=4) as p3o, \
             tc.tile_pool(name="ps2s", bufs=2, space="PSUM") as ps2s, \
             tc.tile_pool(name="ps2c", bufs=1, space="PSUM") as ps2c, \
             tc.tile_pool(name="ps3", bufs=2, space="PSUM") as ps3:
            wo = p3w.tile([128, NKD, D], f32r)
            nc.sync.dma_start(out=wo,
                              in_=w_out.rearrange("(kt p) c -> p kt c", p=128))

            def attention(b):
                for qc in range(NQC):
                    q0 = b * S + qc * QC
                    ctx = ps2c.tile([DH + 1, HPC, QC], f32, tag="ctx")
                    nk = (qc * QC) // 128 + 4
                    jbase = (qc * QC) // 128

                    def av(pend, last):
                        kt, off, pex = pend
                        for s in range(HPC):
                            nc.tensor.matmul(
                                out=ctx[:, s, off:QC],
                                lhsT=vA[:, (b * S) // 128 + kt, s, :],
                                rhs=pex[:, s, off:QC],
                                start=(kt == 0), stop=last)

                    pend = None
                    for kt in range(nk):
                        k0 = b * S + kt * 128
                        j = kt - jbase
                        off = 128 * j if j >= 0 else 0
                        scp = ps2s.tile([128, HPC, QC], f32, tag="sc")
                        # two half-array (K=64) score matmuls run concurrently
                        for s in range(HPC):
                            nc.tensor.matmul(
                                out=scp[:, s, :],
                                lhsT=kT16[DH * s:DH * s + DH, k0:k0 + 128],
                                rhs=qT[DH * s:DH * s + DH, q0:q0 + QC],
                                start=True, stop=True)
                        ex2 = p2.tile([128, HPC, QC], bf16, tag="ex")
                        nc.scalar.activation(out=ex2[:, :, off:QC],
                                             in_=scp[:, :, off:QC], func=EXP)
                        if j >= 0:
                            for s in range(HPC):
                                nc.vector.tensor_mul(
                                    ex2[:, s, off:QC], ex2[:, s, off:QC],
                                    triE[:, 384:384 + QC - off])
                        if pend is not None:
                            av(pend, False)
                        pend = (kt, off, ex2)
                    av(pend, True)
                    # normalize both heads: bcast sum, fast recip, multiply
                    for s in range(HPC):
                        se = p2s.tile([1, QC], f32, tag="se")
                        nc.vector.tensor_copy(out=se[:, :],
                                              in_=ctx[DH:DH + 1, s, :])
                        bc = p2s.tile([DH, QC], f32, tag="bc")
                        nc.gpsimd.partition_broadcast(bc[:, :], se[0:1, :])
                        nc.vector.reciprocal_approx_fast(out=bc[:, :],
                                                         in_=bc[:, :])
                        cx = p2s.tile([DH, QC], f32r, tag="cx")
                        nc.vector.tensor_mul(cx[:, :], ctx[0:DH, s, :],
                                             bc[:, :])
                        rel = qc * QC
                        for t in range(max(1, QC // W)):
                            j_sh = rel // W + t
                            ww = min(W, QC)
                            nc.gpsimd.dma_start(
                                out=a2a_in[b][128 * j_sh + DH * s:
                                              128 * j_sh + DH * s + DH, :],
                                in_=cx[:, t * ww:(t + 1) * ww])

            def out_proj(b):
                ctxf = p3.tile([128, NKD, W], f32r, tag="ctxf")
                nc.sync.dma_start(
                    out=ctxf,
                    in_=a2a_out[b][:].rearrange("(kt p) r -> p kt r", p=128))
                for rt in range(W // 128):
                    for nch in range(D // 512):
                        ps_o = ps3.tile([128, 512], f32, tag="po")
                        nc.tensor.matmul(
                            out=ps_o[:, :],
                            lhsT=ones_sb[0:1, 0:128],
                            rhs=bout_sb[0:1, nch * 512:(nch + 1) * 512],
                            start=True, stop=False)
                        for kt in range(NKD):
                            nc.tensor.matmul(
                                out=ps_o[:, :],
                                lhsT=ctxf[:, kt, rt * 128:(rt + 1) * 128],
                                rhs=wo[:, kt, nch * 512:(nch + 1) * 512],
                                start=False, stop=(kt == NKD - 1))
                        ob = p3o.tile([128, 512], f32, tag="ob")
                        nc.vector.tensor_copy(out=ob[:, :], in_=ps_o[:, :])
                        nc.gpsimd.dma_start(
                            out=o_out[b, rt * 128:(rt + 1) * 128,
                                      nch * 512:(nch + 1) * 512],
                            in_=ob[:, :])

            for b in range(B):
                attention(b)
                nc.gpsimd.collective_compute(
                    "AllToAll", mybir.AluOpType.bypass, replica_groups=RG,
                    ins=[a2a_in[b][:]], outs=[a2a_out[b][:]])
                if b >= 1:
                    out_proj(b - 1)
            out_proj(B - 1)

    nc.compile()
    return nc


def _get_nc():
    if "nc" not in _CACHE:
        _CACHE["nc"] = _build()
    return _CACHE["nc"]


def _host_inputs(x, w_in, b_in, w_out, b_out):
    """Build the 8 per-core input maps."""
    x = np.asarray(x, dtype=np.float32)
    w_in = np.asarray(w_in, dtype=np.float32)
    b_in = np.asarray(b_in, dtype=np.float32)
    w_out = np.asarray(w_out, dtype=np.float32)
    b_out = np.asarray(b_out, dtype=np.float32)
    Bb, Ss, _ = x.shape
    R = Bb * Ss

    xT = np.ascontiguousarray(x.reshape(R, D).T)
    x16 = xT.astype(ml_dtypes.bfloat16)
    scale = np.float32(1.0 / np.sqrt(DH))

    # causal triangle, extended for the 4 diagonal offsets:
    # triE[kk, u] = 1 iff u >= 384 + kk  (u in [0, 896))
    u = np.arange(896, dtype=np.int32)[None, :]
    kk = np.arange(128, dtype=np.int32)[:, None]
    triE = (u >= 384 + kk).astype(ml_dtypes.bfloat16)

    w_out_c = np.ascontiguousarray(w_out)
    b_out_c = b_out.reshape(1, D).astype(ml_dtypes.bfloat16)

    in_maps = []
    for c in range(NCORES):
        cols = slice(HPC * DH * c, HPC * DH * (c + 1))
        wq = (w_in[:, 0:D][:, cols] * scale).astype(ml_dtypes.bfloat16)
        wk = w_in[:, D:2 * D][:, cols]
        wv = w_in[:, 2 * D:3 * D][:, cols]
        bq = b_in[0:D][cols] * scale
        bk = b_in[D:2 * D][cols]
        bv = b_in[2 * D:3 * D][cols]
        in_maps.append({
            "xT": xT,
            "x16": x16,
            "w_kv": np.ascontiguousarray(np.concatenate([wk, wv], axis=1)),
            "w_q16": np.ascontiguousarray(wq),
            "b_qkv": np.concatenate([bq, bk, bv]).reshape(1, -1)
                .astype(ml_dtypes.bfloat16),
            "w_out": w_out_c,
            "b_out": b_out_c,
            "tri": triE,
            "cst": np.ones((128, 512), dtype=ml_dtypes.bfloat16),
        })
    return in_maps


def _assemble(results, B=4, S=2048):
    R = B * S
    W = S // NCORES
    k = np.empty((B, H, S, DH), dtype=np.float32)
    v = np.empty((B, H, S, DH), dtype=np.float32)
    out_flat = np.empty((R, D), dtype=np.float32)
    for c in range(NCORES):
        kc = results[c]["kT_out"].reshape(HPC, DH, B, S)
        vc = results[c]["vT_out"].reshape(HPC, DH, B, S)
        k[:, HPC * c:HPC * (c + 1)] = kc.transpose(2, 0, 3, 1)
        v[:, HPC * c:HPC * (c + 1)] = vc.transpose(2, 0, 3, 1)
        oc = results[c]["o_out"]          # [B, W, D]
        for b in range(B):
            g0 = b * S + W * c
            out_flat[g0:g0 + W] = oc[b]
    out = out_flat.reshape(B, S, D)
    return out, (k, v)


def kernel(x, w_in, b_in, w_out, b_out):
    from concourse.bass_utils import run_bass_kernel_spmd
    nc = _get_nc()
    in_maps = _host_inputs(x, w_in, b_in, w_out, b_out)
    res = run_bass_kernel_spmd(nc, in_maps, list(range(NCORES)))
    return _assemble(res.results)


# revision 10
# speedup vs baseline: 1.0312x; 1.0312x over previous
"""Multi-head causal attention (B=4, S=2048, D=1024, H=16) on 8 Trainium2 cores.

Strategy: tensor-parallel over heads (2 heads/core).
 - Host feeds each core xT = x^T [D, B*S] (fp32r + bf16 copies) plus that
   core's slice of w_in columns (q cols pre-scaled by 1/sqrt(dh)), and full
   w_out.
 - Phase 1: k,v chains in fp32r (output precision), q chain in bf16;
   qT/kT resident in SBUF; vT staged + PE-transposed to v-natural bf16 tiles
   with a ones column appended (flash-attention sum trick).
 - Phase 2 (bf16 matmuls): per (batch, head): scoresT[k,q] pairs of k-tiles
   into one 2-bank PSUM tile, one Exp per pair (ACT), causal mask multiply,
   ctxT[dh+1, q] accumulated on PE with v_aug stationary; row dh = sum(exp).
   Software-pipelined (next pair's scores before this pair's AV matmuls).
   Normalize via partition_broadcast + reciprocal_approx_fast.
 - Four AllToAlls (one per batch) reshard ctxT from head-split to row-split;
   all overlap attention/out-proj compute.
 - Phase 3: out rows-slice = ctxT_full^T @ w_out + b_out (fp32r).
Outputs per core: kT/vT head slices and out rows-slices; host reassembles.
"""

import numpy as np
import ml_dtypes
from contextlib import ExitStack

NCORES = 8
DH = 64
H = 16
HPC = H // NCORES          # heads per core = 2
D = H * DH                 # 1024
NKD = D // 128             # 8 contraction tiles over D
RC = 512                   # phase-1 row chunk
QC = 512                   # phase-2 query chunk

_CACHE = {}


def _build(B=4, S=2048):
    import concourse.tile as tile
    from concourse import bacc, mybir
    from concourse.masks import make_identity

    R = B * S
    W = S // NCORES        # per-batch A2A shard width (rows)
    NQC = S // QC
    f32 = mybir.dt.float32
    f32r = mybir.dt.float32r
    bf16 = mybir.dt.bfloat16
    EXP = mybir.ActivationFunctionType.Exp

    nc = bacc.Bacc("TRN2", target_bir_lowering=False, debug=False,
                   num_devices=NCORES)

    xT = nc.dram_tensor("xT", [D, R], f32r, kind="ExternalInput").ap()
    x16 = nc.dram_tensor("x16", [D, R], bf16, kind="ExternalInput").ap()
    w_kv = nc.dram_tensor("w_kv", [D, 2 * HPC * DH], f32r,
                          kind="ExternalInput").ap()
    w_q16 = nc.dram_tensor("w_q16", [D, HPC * DH], bf16,
                           kind="ExternalInput").ap()
    b_qkv = nc.dram_tensor("b_qkv", [1, 3 * HPC * DH], bf16,
                           kind="ExternalInput").ap()
    w_out = nc.dram_tensor("w_out", [D, D], f32r, kind="ExternalInput").ap()
    b_out = nc.dram_tensor("b_out", [1, D], bf16, kind="ExternalInput").ap()
    tri = nc.dram_tensor("tri", [128, 896], bf16, kind="ExternalInput").ap()
    cst = nc.dram_tensor("cst", [128, 512], bf16, kind="ExternalInput").ap()

    kT_out = nc.dram_tensor("kT_out", [HPC * DH, R], f32,
                            kind="ExternalOutput").ap()
    vT_out = nc.dram_tensor("vT_out", [HPC * DH, R], f32,
                            kind="ExternalOutput").ap()
    o_out = nc.dram_tensor("o_out", [B, W, D], f32,
                           kind="ExternalOutput").ap()

    # per-batch A2A buffers
    a2a_in = [nc.dram_tensor(f"a2a_in{b}", [NCORES * 128, W], f32r)
              for b in range(B)]
    a2a_out = [nc.dram_tensor(f"a2a_out{b}", [NCORES * 128, W], f32r)
               for b in range(B)]
    RG = [list(range(NCORES))]

    with tile.TileContext(nc) as tc, ExitStack() as ctx:
        persist = ctx.enter_context(tc.tile_pool(name="persist", bufs=1))
        qT = persist.tile([128, R], bf16)
        kT = persist.tile([128, R], f32)
        kT16 = persist.tile([128, R], bf16)
        vA = persist.tile([128, R // 128, HPC, DH + 1], bf16)
        triE = persist.tile([128, 896], bf16)
        ident = persist.tile([128, 128], f32)
        ones_sb = persist.tile([128, 512], bf16)
        bout_sb = persist.tile([1, D], bf16)

        make_identity(nc, ident)
        nc.sync.dma_start(out=ones_sb, in_=cst)
        nc.sync.dma_start(out=triE, in_=tri)
        nc.sync.dma_start(out=bout_sb, in_=b_out)
        # ones column of vA via one strided copy (the v copies in phase 1
        # fill the rest).
        nc.vector.tensor_copy(
            out=vA[:, :, :, DH:DH + 1],
            in_=ones_sb[:, 0:R // 128 * HPC].rearrange(
                "p (a b c) -> p a b c", b=HPC, c=1))

        # ---------------- Phase 1: qkv projection ----------------
        with tc.tile_pool(name="p1", bufs=3) as p1, \
             tc.tile_pool(name="p1w", bufs=1) as p1w, \
             tc.tile_pool(name="ps1", bufs=2, space="PSUM") as ps1:
            wkv = p1w.tile([128, NKD, 2 * HPC * DH], f32r)
            nc.sync.dma_start(out=wkv,
                              in_=w_kv.rearrange("(kt p) c -> p kt c", p=128))
            wq16 = p1w.tile([128, NKD, HPC * DH], bf16)
            nc.sync.dma_start(out=wq16,
                              in_=w_q16.rearrange("(kt p) c -> p kt c", p=128))
            bq = p1w.tile([1, 3 * HPC * DH], bf16)
            nc.sync.dma_start(out=bq, in_=b_qkv)

            for rc in range(R // RC):
                r0 = rc * RC
                xt = p1.tile([128, NKD, RC], f32r, tag="xt")
                nc.sync.dma_start(
                    out=xt,
                    in_=xT[:, r0:r0 + RC].rearrange("(kt p) r -> p kt r", p=128))
                xt16 = p1.tile([128, NKD, RC], bf16, tag="xt16")
                nc.scalar.dma_start(
                    out=xt16,
                    in_=x16[:, r0:r0 + RC].rearrange("(kt p) r -> p kt r",
                                                     p=128))
                ps_q = ps1.tile([128, RC], f32, tag="psq")
                ps_k = ps1.tile([128, RC], f32, tag="psk")
                ps_v = ps1.tile([128, RC], f32, tag="psv")
                # q chain in bf16
                nc.tensor.matmul(out=ps_q[:, :], lhsT=bq[0:1, 0:128],
                                 rhs=ones_sb[0:1, 0:RC], start=True, stop=False)
                for kt in range(NKD):
                    nc.tensor.matmul(out=ps_q[:, :], lhsT=wq16[:, kt, :],
                                     rhs=xt16[:, kt, :],
                                     start=False, stop=(kt == NKD - 1))
                # k, v chains in fp32r
                for ps_t, c0 in ((ps_k, 0), (ps_v, 128)):
                    nc.tensor.matmul(out=ps_t[:, :],
                                     lhsT=bq[0:1, 128 + c0:256 + c0],
                                     rhs=ones_sb[0:1, 0:RC],
                                     start=True, stop=False)
                    for kt in range(NKD):
                        nc.tensor.matmul(out=ps_t[:, :],
                                         lhsT=wkv[:, kt, c0:c0 + 128],
                                         rhs=xt[:, kt, :],
                                         start=False, stop=(kt == NKD - 1))
                nc.vector.tensor_copy(out=qT[:, r0:r0 + RC], in_=ps_q[:, :])
                nc.vector.tensor_copy(out=kT[:, r0:r0 + RC], in_=ps_k[:, :])
                nc.vector.tensor_copy(out=kT16[:, r0:r0 + RC], in_=ps_k[:, :])
                nc.sync.dma_start(out=kT_out[:, r0:r0 + RC],
                                  in_=kT[:, r0:r0 + RC])
                vt_sb = p1.tile([128, RC], f32, tag="vtsb")
                nc.vector.tensor_copy(out=vt_sb[:, :], in_=ps_v[:, :])
                nc.sync.dma_start(out=vT_out[:, r0:r0 + RC],
                                  in_=vt_sb[:, :])
                for t4 in range(RC // 128):
                    ps_vt = ps1.tile([128, 128], f32, tag="psvt")
                    nc.tensor.transpose(ps_vt[:, :],
                                        vt_sb[:, t4 * 128:(t4 + 1) * 128],
                                        ident)
                    rt = rc * (RC // 128) + t4
                    for hh in range(HPC):
                        nc.vector.tensor_copy(
                            out=vA[:, rt, hh, 0:DH],
                            in_=ps_vt[:, hh * DH:(hh + 1) * DH])

        # ---- Phase 2 (attention, heads packed as concurrent row-tiles) ----
        # ---- interleaved with per-batch A2A + Phase 3 out-proj ----
        with tc.tile_pool(name="p2", bufs=4) as p2, \
             tc.tile_pool(name="p2s", bufs=2) as p2s, \
             tc.tile_pool(name="p3", bufs=2) as p3, \
             tc.tile_pool(name="p3w", bufs=1) as p3w, \
             tc.tile_pool(name="p3o", bufs=4) as p3o, \
             tc.tile_pool(name="ps2s", bufs=2, space="PSUM") as ps2s, \
             tc.tile_pool(name="ps2c", bufs=1, space="PSUM") as ps2c, \
             tc.tile_pool(name="ps3", bufs=2, space="PSUM") as ps3:
            wo = p3w.tile([128, NKD, D], f32r)
            nc.sync.dma_start(out=wo,
                              in_=w_out.rearrange("(kt p) c -> p kt c", p=128))

            def attention(b):
                for qc in range(NQC):
                    q0 = b * S + qc * QC
                    ctx = ps2c.tile([DH + 1, HPC, QC], f32, tag="ctx")
                    nk = (qc * QC) // 128 + 4
                    jbase = (qc * QC) // 128

                    def av(pend, last):
                        kt, off, pex = pend
                        for s in range(HPC):
                            nc.tensor.matmul(
                                out=ctx[:, s, off:QC],
                                lhsT=vA[:, (b * S) // 128 + kt, s, :],
                                rhs=pex[:, s, off:QC],
                                start=(kt == 0), stop=last)

                    pend = None
                    for kt in range(nk):
                        k0 = b * S + kt * 128
                        j = kt - jbase
                        off = 128 * j if j >= 0 else 0
                        scp = ps2s.tile([128, HPC, QC], f32, tag="sc")
                        # two half-array (K=64) score matmuls run concurrently
                        for s in range(HPC):
                            nc.tensor.matmul(
                                out=scp[:, s, :],
                                lhsT=kT16[DH * s:DH * s + DH, k0:k0 + 128],
                                rhs=qT[DH * s:DH * s + DH, q0:q0 + QC],
                                start=True, stop=True)
                        ex2 = p2.tile([128, HPC, QC], bf16, tag="ex")
                        nc.scalar.activation(out=ex2[:, :, off:QC],
                                             in_=scp[:, :, off:QC], func=EXP)
                        if j >= 0:
                            for s in range(HPC):
                                nc.vector.tensor_mul(
                                    ex2[:, s, off:QC], ex2[:, s, off:QC],
                                    triE[:, 384:384 + QC - off])
                        if pend is not None:
                            av(pend, False)
                        pend = (kt, off, ex2)
                    av(pend, True)
                    # normalize both heads: bcast sum, fast recip, multiply
                    for s in range(HPC):
                        se = p2s.tile([1, QC], f32, tag="se")
                        nc.vector.tensor_copy(out=se[:, :],
                                              in_=ctx[DH:DH + 1, s, :])
                        bc = p2s.tile([DH, QC], f32, tag="bc")
                        nc.gpsimd.partition_broadcast(bc[:, :], se[0:1, :])
                        nc.vector.reciprocal_approx_fast(out=bc[:, :],
                                                         in_=bc[:, :])
                        cx = p2s.tile([DH, QC], f32r, tag="cx")
                        nc.vector.tensor_mul(cx[:, :], ctx[0:DH, s, :],
                                             bc[:, :])
                        rel = qc * QC
                        for t in range(max(1, QC // W)):
                            j_sh = rel // W + t
                            ww = min(W, QC)
                            nc.sync.dma_start(
                                out=a2a_in[b][128 * j_sh + DH * s:
                                              128 * j_sh + DH * s + DH, :],
                                in_=cx[:, t * ww:(t + 1) * ww])

            def out_proj(b):
                ctxf = p3.tile([128, NKD, W], f32r, tag="ctxf")
                nc.sync.dma_start(
                    out=ctxf,
                    in_=a2a_out[b][:].rearrange("(kt p) r -> p kt r", p=128))
                for rt in range(W // 128):
                    for nch in range(D // 512):
                        ps_o = ps3.tile([128, 512], f32, tag="po")
                        nc.tensor.matmul(
                            out=ps_o[:, :],
                            lhsT=ones_sb[0:1, 0:128],
                            rhs=bout_sb[0:1, nch * 512:(nch + 1) * 512],
                            start=True, stop=False)
                        for kt in range(NKD):
                            nc.tensor.matmul(
                                out=ps_o[:, :],
                                lhsT=ctxf[:, kt, rt * 128:(rt + 1) * 128],
                                rhs=wo[:, kt, nch * 512:(nch + 1) * 512],
                                start=False, stop=(kt == NKD - 1))
                        ob = p3o.tile([128, 512], f32, tag="ob")
                        nc.vector.tensor_copy(out=ob[:, :], in_=ps_o[:, :])
                        nc.sync.dma_start(
                            out=o_out[b, rt * 128:(rt + 1) * 128,
                                      nch * 512:(nch + 1) * 512],
                            in_=ob[:, :])

            for b in range(B):
                attention(b)
                nc.gpsimd.collective_compute(
                    "AllToAll", mybir.AluOpType.bypass, replica_groups=RG,
                    ins=[a2a_in[b][:]], outs=[a2a_out[b][:]])
                if b >= 1:
                    out_proj(b - 1)
            out_proj(B - 1)

    nc.compile()
    return nc


def _get_nc():
    if "nc" not in _CACHE:
        _CACHE["nc"] = _build()
    return _CACHE["nc"]


def _host_inputs(x, w_in, b_in, w_out, b_out):
    """Build the 8 per-core input maps."""
    x = np.asarray(x, dtype=np.float32)
    w_in = np.asarray(w_in, dtype=np.float32)
    b_in = np.asarray(b_in, dtype=np.float32)
    w_out = np.asarray(w_out, dtype=np.float32)
    b_out = np.asarray(b_out, dtype=np.float32)
    Bb, Ss, _ = x.shape
    R = Bb * Ss

    xT = np.ascontiguousarray(x.reshape(R, D).T)
    x16 = xT.astype(ml_dtypes.bfloat16)
    scale = np.float32(1.0 / np.sqrt(DH))

    # causal triangle, extended for the 4 diagonal offsets:
    # triE[kk, u] = 1 iff u >= 384 + kk  (u in [0, 896))
    u = np.arange(896, dtype=np.int32)[None, :]
    kk = np.arange(128, dtype=np.int32)[:, None]
    triE = (u >= 384 + kk).astype(ml_dtypes.bfloat16)

    w_out_c = np.ascontiguousarray(w_out)
    b_out_c = b_out.reshape(1, D).astype(ml_dtypes.bfloat16)

    in_maps = []
    for c in range(NCORES):
        cols = slice(HPC * DH * c, HPC * DH * (c + 1))
        wq = (w_in[:, 0:D][:, cols] * scale).astype(ml_dtypes.bfloat16)
        wk = w_in[:, D:2 * D][:, cols]
        wv = w_in[:, 2 * D:3 * D][:, cols]
        bq = b_in[0:D][cols] * scale
        bk = b_in[D:2 * D][cols]
        bv = b_in[2 * D:3 * D][cols]
        in_maps.append({
            "xT": xT,
            "x16": x16,
            "w_kv": np.ascontiguousarray(np.concatenate([wk, wv], axis=1)),
            "w_q16": np.ascontiguousarray(wq),
            "b_qkv": np.concatenate([bq, bk, bv]).reshape(1, -1)
                .astype(ml_dtypes.bfloat16),
            "w_out": w_out_c,
            "b_out": b_out_c,
            "tri": triE,
            "cst": np.ones((128, 512), dtype=ml_dtypes.bfloat16),
        })
    return in_maps


def _assemble(results, B=4, S=2048):
    R = B * S
    W = S // NCORES
    k = np.empty((B, H, S, DH), dtype=np.float32)
    v = np.empty((B, H, S, DH), dtype=np.float32)
    out_flat = np.empty((R, D), dtype=np.float32)
    for c in range(NCORES):
        kc = results[c]["kT_out"].reshape(HPC, DH, B, S)
        vc = results[c]["vT_out"].reshape(HPC, DH, B, S)
        k[:, HPC * c:HPC * (c + 1)] = kc.transpose(2, 0, 3, 1)
        v[:, HPC * c:HPC * (c + 1)] = vc.transpose(2, 0, 3, 1)
        oc = results[c]["o_out"]          # [B, W, D]
        for b in range(B):
            g0 = b * S + W * c
            out_flat[g0:g0 + W] = oc[b]
    out = out_flat.reshape(B, S, D)
    return out, (k, v)


def kernel(x, w_in, b_in, w_out, b_out):
    from concourse.bass_utils import run_bass_kernel_spmd
    nc = _get_nc()
    in_maps = _host_inputs(x, w_in, b_in, w_out, b_out)
    res = run_bass_kernel_spmd(nc, in_maps, list(range(NCORES)))
    return _assemble(res.results)


# revision 13
# speedup vs baseline: 1.0709x; 1.0385x over previous
"""Multi-head causal attention (B=4, S=2048, D=1024, H=16) on 8 Trainium2 cores.

Strategy: tensor-parallel over heads (2 heads/core).
 - Host feeds each core xT = x^T [D, B*S] (fp32r + bf16 copies) plus that
   core's slice of w_in columns (q cols pre-scaled by 1/sqrt(dh)), and full
   w_out.
 - Phase 1: k,v chains in fp32r (output precision), q chain in bf16;
   qT/kT resident in SBUF; vT staged + PE-transposed to v-natural bf16 tiles
   with a ones column appended (flash-attention sum trick).
 - Phase 2 (bf16 matmuls): per (batch, head): scoresT[k,q] pairs of k-tiles
   into one 2-bank PSUM tile, one Exp per pair (ACT), causal mask multiply,
   ctxT[dh+1, q] accumulated on PE with v_aug stationary; row dh = sum(exp).
   Software-pipelined (next pair's scores before this pair's AV matmuls).
   Normalize via partition_broadcast + reciprocal_approx_fast.
 - Four AllToAlls (one per batch) reshard ctxT from head-split to row-split;
   all overlap attention/out-proj compute.
 - Phase 3: out rows-slice = ctxT_full^T @ w_out + b_out (fp32r).
Outputs per core: kT/vT head slices and out rows-slices; host reassembles.
"""

import numpy as np
import ml_dtypes
from contextlib import ExitStack

NCORES = 8
DH = 64
H = 16
HPC = H // NCORES          # heads per core = 2
D = H * DH                 # 1024
NKD = D // 128             # 8 contraction tiles over D
RC = 512                   # phase-1 row chunk
QC = 512                   # phase-2 query chunk

_CACHE = {}


def _build(B=4, S=2048):
    import concourse.tile as tile
    from concourse import bacc, mybir
    from concourse.masks import make_identity

    R = B * S
    W = S // NCORES        # per-batch A2A shard width (rows)
    NQC = S // QC
    f32 = mybir.dt.float32
    f32r = mybir.dt.float32r
    bf16 = mybir.dt.bfloat16
    EXP = mybir.ActivationFunctionType.Exp

    nc = bacc.Bacc("TRN2", target_bir_lowering=False, debug=False,
                   num_devices=NCORES)

    xT = nc.dram_tensor("xT", [D, R], f32r, kind="ExternalInput").ap()
    w_qkv = nc.dram_tensor("w_qkv", [D, 3 * HPC * DH], f32r,
                           kind="ExternalInput").ap()
    b_qkv = nc.dram_tensor("b_qkv", [1, 3 * HPC * DH], bf16,
                           kind="ExternalInput").ap()
    w_out = nc.dram_tensor("w_out", [D, D], f32r, kind="ExternalInput").ap()
    b_out = nc.dram_tensor("b_out", [1, D], bf16, kind="ExternalInput").ap()
    tri = nc.dram_tensor("tri", [128, 896], bf16, kind="ExternalInput").ap()
    cst = nc.dram_tensor("cst", [128, 512], bf16, kind="ExternalInput").ap()

    kT_out = nc.dram_tensor("kT_out", [HPC * DH, R], f32,
                            kind="ExternalOutput").ap()
    vT_out = nc.dram_tensor("vT_out", [HPC * DH, R], f32,
                            kind="ExternalOutput").ap()
    o_out = nc.dram_tensor("o_out", [B, W, D], f32,
                           kind="ExternalOutput").ap()

    # per-batch A2A buffers
    a2a_in = [nc.dram_tensor(f"a2a_in{b}", [NCORES * 128, W], f32r)
              for b in range(B)]
    a2a_out = [nc.dram_tensor(f"a2a_out{b}", [NCORES * 128, W], f32r)
               for b in range(B)]
    RG = [list(range(NCORES))]

    with tile.TileContext(nc) as tc, ExitStack() as ctx:
        persist = ctx.enter_context(tc.tile_pool(name="persist", bufs=1))
        qT = persist.tile([128, R], bf16)
        kT = persist.tile([128, R], f32)
        kT16 = persist.tile([128, R], bf16)
        vA = persist.tile([128, R // 128, HPC, DH + 1], bf16)
        triE = persist.tile([128, 896], bf16)
        ident = persist.tile([128, 128], f32)
        ones_sb = persist.tile([128, 512], bf16)
        bout_sb = persist.tile([1, D], bf16)

        make_identity(nc, ident)
        nc.sync.dma_start(out=ones_sb, in_=cst)
        nc.sync.dma_start(out=triE, in_=tri)
        nc.sync.dma_start(out=bout_sb, in_=b_out)
        # ones column of vA via one strided copy (the v copies in phase 1
        # fill the rest).
        nc.vector.tensor_copy(
            out=vA[:, :, :, DH:DH + 1],
            in_=ones_sb[:, 0:R // 128 * HPC].rearrange(
                "p (a b c) -> p a b c", b=HPC, c=1))

        # ---------------- Phase 1: qkv projection ----------------
        with tc.tile_pool(name="p1", bufs=3) as p1, \
             tc.tile_pool(name="p1w", bufs=1) as p1w, \
             tc.tile_pool(name="ps1", bufs=2, space="PSUM") as ps1:
            wq = p1w.tile([128, NKD, 3 * HPC * DH], f32r)
            nc.sync.dma_start(out=wq,
                              in_=w_qkv.rearrange("(kt p) c -> p kt c", p=128))
            bq = p1w.tile([1, 3 * HPC * DH], bf16)
            nc.sync.dma_start(out=bq, in_=b_qkv)

            for rc in range(R // RC):
                r0 = rc * RC
                xt = p1.tile([128, NKD, RC], f32r, tag="xt")
                nc.sync.dma_start(
                    out=xt,
                    in_=xT[:, r0:r0 + RC].rearrange("(kt p) r -> p kt r", p=128))
                ps_q = ps1.tile([128, RC], f32, tag="psq")
                ps_k = ps1.tile([128, RC], f32, tag="psk")
                ps_v = ps1.tile([128, RC], f32, tag="psv")
                for ps_t, c0 in ((ps_q, 0), (ps_k, 128), (ps_v, 256)):
                    nc.tensor.matmul(out=ps_t[:, :],
                                     lhsT=bq[0:1, c0:c0 + 128],
                                     rhs=ones_sb[0:1, 0:RC],
                                     start=True, stop=False)
                    for kt in range(NKD):
                        nc.tensor.matmul(out=ps_t[:, :],
                                         lhsT=wq[:, kt, c0:c0 + 128],
                                         rhs=xt[:, kt, :],
                                         start=False, stop=(kt == NKD - 1))
                nc.vector.tensor_copy(out=qT[:, r0:r0 + RC], in_=ps_q[:, :])
                nc.vector.tensor_copy(out=kT[:, r0:r0 + RC], in_=ps_k[:, :])
                nc.vector.tensor_copy(out=kT16[:, r0:r0 + RC], in_=ps_k[:, :])
                nc.sync.dma_start(out=kT_out[:, r0:r0 + RC],
                                  in_=kT[:, r0:r0 + RC])
                vt_sb = p1.tile([128, RC], f32, tag="vtsb")
                nc.vector.tensor_copy(out=vt_sb[:, :], in_=ps_v[:, :])
                nc.sync.dma_start(out=vT_out[:, r0:r0 + RC],
                                  in_=vt_sb[:, :])
                for t4 in range(RC // 128):
                    ps_vt = ps1.tile([128, 128], f32, tag="psvt")
                    nc.tensor.transpose(ps_vt[:, :],
                                        vt_sb[:, t4 * 128:(t4 + 1) * 128],
                                        ident)
                    rt = rc * (RC // 128) + t4
                    for hh in range(HPC):
                        nc.vector.tensor_copy(
                            out=vA[:, rt, hh, 0:DH],
                            in_=ps_vt[:, hh * DH:(hh + 1) * DH])

        # ---- Phase 2 (attention, heads packed as concurrent row-tiles) ----
        # ---- interleaved with per-batch A2A + Phase 3 out-proj ----
        with tc.tile_pool(name="p2", bufs=4) as p2, \
             tc.tile_pool(name="p2s", bufs=2) as p2s, \
             tc.tile_pool(name="p3", bufs=2) as p3, \
             tc.tile_pool(name="p3w", bufs=1) as p3w, \
             tc.tile_pool(name="p3o", bufs=4) as p3o, \
             tc.tile_pool(name="ps2s", bufs=2, space="PSUM") as ps2s, \
             tc.tile_pool(name="ps2c", bufs=1, space="PSUM") as ps2c, \
             tc.tile_pool(name="ps3", bufs=2, space="PSUM") as ps3:
            wo = p3w.tile([128, NKD, D], f32r)
            nc.sync.dma_start(out=wo,
                              in_=w_out.rearrange("(kt p) c -> p kt c", p=128))

            def attention(b):
                for qc in range(NQC):
                    q0 = b * S + qc * QC
                    ctx = ps2c.tile([DH + 1, HPC, QC], f32, tag="ctx")
                    nk = (qc * QC) // 128 + 4
                    jbase = (qc * QC) // 128

                    def av(pend, last):
                        kt, off, pex = pend
                        for s in range(HPC):
                            nc.tensor.matmul(
                                out=ctx[:, s, off:QC],
                                lhsT=vA[:, (b * S) // 128 + kt, s, :],
                                rhs=pex[:, s, off:QC],
                                start=(kt == 0), stop=last)

                    pend = None
                    for kt in range(nk):
                        k0 = b * S + kt * 128
                        j = kt - jbase
                        off = 128 * j if j >= 0 else 0
                        scp = ps2s.tile([128, HPC, QC], f32, tag="sc")
                        # two half-array (K=64) score matmuls run concurrently
                        for s in range(HPC):
                            nc.tensor.matmul(
                                out=scp[:, s, :],
                                lhsT=kT16[DH * s:DH * s + DH, k0:k0 + 128],
                                rhs=qT[DH * s:DH * s + DH, q0:q0 + QC],
                                start=True, stop=True)
                        ex2 = p2.tile([128, HPC, QC], bf16, tag="ex")
                        nc.scalar.activation(out=ex2[:, :, off:QC],
                                             in_=scp[:, :, off:QC], func=EXP)
                        if j >= 0:
                            for s in range(HPC):
                                nc.vector.tensor_mul(
                                    ex2[:, s, off:QC], ex2[:, s, off:QC],
                                    triE[:, 384:384 + QC - off])
                        if pend is not None:
                            av(pend, False)
                        pend = (kt, off, ex2)
                    av(pend, True)
                    # normalize both heads: bcast sum, fast recip, multiply
                    for s in range(HPC):
                        se = p2s.tile([1, QC], f32, tag="se")
                        nc.vector.tensor_copy(out=se[:, :],
                                              in_=ctx[DH:DH + 1, s, :])
                        bc = p2s.tile([DH, QC], f32, tag="bc")
                        nc.gpsimd.partition_broadcast(bc[:, :], se[0:1, :])
                        nc.vector.reciprocal_approx_fast(out=bc[:, :],
                                                         in_=bc[:, :])
                        cx = p2s.tile([DH, QC], f32r, tag="cx")
                        nc.vector.tensor_mul(cx[:, :], ctx[0:DH, s, :],
                                             bc[:, :])
                        rel = qc * QC
                        for t in range(max(1, QC // W)):
                            j_sh = rel // W + t
                            ww = min(W, QC)
                            nc.sync.dma_start(
                                out=a2a_in[b][128 * j_sh + DH * s:
                                              128 * j_sh + DH * s + DH, :],
                                in_=cx[:, t * ww:(t + 1) * ww])

            def out_proj(b):
                ctxf = p3.tile([128, NKD, W], f32r, tag="ctxf")
                nc.sync.dma_start(
                    out=ctxf,
                    in_=a2a_out[b][:].rearrange("(kt p) r -> p kt r", p=128))
                for rt in range(W // 128):
                    for nch in range(D // 512):
                        ps_o = ps3.tile([128, 512], f32, tag="po")
                        nc.tensor.matmul(
                            out=ps_o[:, :],
                            lhsT=ones_sb[0:1, 0:128],
                            rhs=bout_sb[0:1, nch * 512:(nch + 1) * 512],
                            start=True, stop=False)
                        for kt in range(NKD):
                            nc.tensor.matmul(
                                out=ps_o[:, :],
                                lhsT=ctxf[:, kt, rt * 128:(rt + 1) * 128],
                                rhs=wo[:, kt, nch * 512:(nch + 1) * 512],
                                start=False, stop=(kt == NKD - 1))
                        ob = p3o.tile([128, 512], f32, tag="ob")
                        nc.vector.tensor_copy(out=ob[:, :], in_=ps_o[:, :])
                        nc.sync.dma_start(
                            out=o_out[b, rt * 128:(rt + 1) * 128,
                                      nch * 512:(nch + 1) * 512],
                            in_=ob[:, :])

            for b in range(B):
                attention(b)
                if b >= 2:
                    out_proj(b - 2)
                nc.gpsimd.collective_compute(
                    "AllToAll", mybir.AluOpType.bypass, replica_groups=RG,
                    ins=[a2a_in[b][:]], outs=[a2a_out[b][:]])
            out_proj(B - 2)
            out_proj(B - 1)

    nc.compile()
    return nc


def _get_nc():
    if "nc" not in _CACHE:
        _CACHE["nc"] = _build()
    return _CACHE["nc"]


def _host_inputs(x, w_in, b_in, w_out, b_out):
    """Build the 8 per-core input maps."""
    x = np.asarray(x, dtype=np.float32)
    w_in = np.asarray(w_in, dtype=np.float32)
    b_in = np.asarray(b_in, dtype=np.float32)
    w_out = np.asarray(w_out, dtype=np.float32)
    b_out = np.asarray(b_out, dtype=np.float32)
    Bb, Ss, _ = x.shape
    R = Bb * Ss

    xT = np.ascontiguousarray(x.reshape(R, D).T)
    scale = np.float32(1.0 / np.sqrt(DH))

    # causal triangle, extended for the 4 diagonal offsets:
    # triE[kk, u] = 1 iff u >= 384 + kk  (u in [0, 896))
    u = np.arange(896, dtype=np.int32)[None, :]
    kk = np.arange(128, dtype=np.int32)[:, None]
    triE = (u >= 384 + kk).astype(ml_dtypes.bfloat16)

    w_out_c = np.ascontiguousarray(w_out)
    b_out_c = b_out.reshape(1, D).astype(ml_dtypes.bfloat16)

    in_maps = []
    for c in range(NCORES):
        cols = slice(HPC * DH * c, HPC * DH * (c + 1))
        wq = w_in[:, 0:D][:, cols] * scale
        wk = w_in[:, D:2 * D][:, cols]
        wv = w_in[:, 2 * D:3 * D][:, cols]
        bq = b_in[0:D][cols] * scale
        bk = b_in[D:2 * D][cols]
        bv = b_in[2 * D:3 * D][cols]
        in_maps.append({
            "xT": xT,
            "w_qkv": np.ascontiguousarray(
                np.concatenate([wq, wk, wv], axis=1)),
            "b_qkv": np.concatenate([bq, bk, bv]).reshape(1, -1)
                .astype(ml_dtypes.bfloat16),
            "w_out": w_out_c,
            "b_out": b_out_c,
            "tri": triE,
            "cst": np.ones((128, 512), dtype=ml_dtypes.bfloat16),
        })
    return in_maps


def _assemble(results, B=4, S=2048):
    R = B * S
    W = S // NCORES
    k = np.empty((B, H, S, DH), dtype=np.float32)
    v = np.empty((B, H, S, DH), dtype=np.float32)
    out_flat = np.empty((R, D), dtype=np.float32)
    for c in range(NCORES):
        kc = results[c]["kT_out"].reshape(HPC, DH, B, S)
        vc = results[c]["vT_out"].reshape(HPC, DH, B, S)
        k[:, HPC * c:HPC * (c + 1)] = kc.transpose(2, 0, 3, 1)
        v[:, HPC * c:HPC * (c + 1)] = vc.transpose(2, 0, 3, 1)
        oc = results[c]["o_out"]          # [B, W, D]
        for b in range(B):
            g0 = b * S + W * c
            out_flat[g0:g0 + W] = oc[b]
    out = out_flat.reshape(B, S, D)
    return out, (k, v)


def kernel(x, w_in, b_in, w_out, b_out):
    from concourse.bass_utils import run_bass_kernel_spmd
    nc = _get_nc()
    in_maps = _host_inputs(x, w_in, b_in, w_out, b_out)
    res = run_bass_kernel_spmd(nc, in_maps, list(range(NCORES)))
    return _assemble(res.results)


# revision 14
# speedup vs baseline: 1.0965x; 1.0239x over previous
"""Multi-head causal attention (B=4, S=2048, D=1024, H=16) on 8 Trainium2 cores.

Strategy: tensor-parallel over heads (2 heads/core).
 - Host feeds each core xT = x^T [D, B*S] (fp32r + bf16 copies) plus that
   core's slice of w_in columns (q cols pre-scaled by 1/sqrt(dh)), and full
   w_out.
 - Phase 1: k,v chains in fp32r (output precision), q chain in bf16;
   qT/kT resident in SBUF; vT staged + PE-transposed to v-natural bf16 tiles
   with a ones column appended (flash-attention sum trick).
 - Phase 2 (bf16 matmuls): per (batch, head): scoresT[k,q] pairs of k-tiles
   into one 2-bank PSUM tile, one Exp per pair (ACT), causal mask multiply,
   ctxT[dh+1, q] accumulated on PE with v_aug stationary; row dh = sum(exp).
   Software-pipelined (next pair's scores before this pair's AV matmuls).
   Normalize via partition_broadcast + reciprocal_approx_fast.
 - Four AllToAlls (one per batch) reshard ctxT from head-split to row-split;
   all overlap attention/out-proj compute.
 - Phase 3: out rows-slice = ctxT_full^T @ w_out + b_out (fp32r).
Outputs per core: kT/vT head slices and out rows-slices; host reassembles.
"""

import numpy as np
import ml_dtypes
from contextlib import ExitStack

NCORES = 8
DH = 64
H = 16
HPC = H // NCORES          # heads per core = 2
D = H * DH                 # 1024
NKD = D // 128             # 8 contraction tiles over D
RC = 512                   # phase-1 row chunk
QC = 512                   # phase-2 query chunk

_CACHE = {}


def _build(B=4, S=2048):
    import concourse.tile as tile
    from concourse import bacc, mybir
    from concourse.masks import make_identity

    R = B * S
    W = S // NCORES        # per-batch A2A shard width (rows)
    NQC = S // QC
    f32 = mybir.dt.float32
    f32r = mybir.dt.float32r
    bf16 = mybir.dt.bfloat16
    EXP = mybir.ActivationFunctionType.Exp

    nc = bacc.Bacc("TRN2", target_bir_lowering=False, debug=False,
                   num_devices=NCORES)

    xT = nc.dram_tensor("xT", [D, R], f32r, kind="ExternalInput").ap()
    w_qkv = nc.dram_tensor("w_qkv", [D, 3 * HPC * DH], f32r,
                           kind="ExternalInput").ap()
    b_qkv = nc.dram_tensor("b_qkv", [1, 3 * HPC * DH], bf16,
                           kind="ExternalInput").ap()
    w_out = nc.dram_tensor("w_out", [D, D], f32r, kind="ExternalInput").ap()
    b_out = nc.dram_tensor("b_out", [1, D], bf16, kind="ExternalInput").ap()
    tri = nc.dram_tensor("tri", [128, 896], bf16, kind="ExternalInput").ap()
    cst = nc.dram_tensor("cst", [128, 512], bf16, kind="ExternalInput").ap()

    kT_out = nc.dram_tensor("kT_out", [HPC * DH, R], f32,
                            kind="ExternalOutput").ap()
    vT_out = nc.dram_tensor("vT_out", [HPC * DH, R], f32,
                            kind="ExternalOutput").ap()
    o_out = nc.dram_tensor("o_out", [B, W, D], f32,
                           kind="ExternalOutput").ap()

    # per-batch A2A buffers
    a2a_in = [nc.dram_tensor(f"a2a_in{b}", [NCORES * 128, W], f32r)
              for b in range(B)]
    a2a_out = [nc.dram_tensor(f"a2a_out{b}", [NCORES * 128, W], f32r)
               for b in range(B)]
    RG = [list(range(NCORES))]

    with tile.TileContext(nc) as tc, ExitStack() as ctx:
        persist = ctx.enter_context(tc.tile_pool(name="persist", bufs=1))
        qT = persist.tile([128, R], bf16)
        kT = persist.tile([128, R], f32)
        kT16 = persist.tile([128, R], bf16)
        vA = persist.tile([128, R // 128, HPC, DH + 1], bf16)
        triE = persist.tile([128, 896], bf16)
        ident = persist.tile([128, 128], f32)
        ones_sb = persist.tile([128, 512], bf16)
        bout_sb = persist.tile([1, D], bf16)

        make_identity(nc, ident)
        nc.sync.dma_start(out=ones_sb, in_=cst)
        nc.sync.dma_start(out=triE, in_=tri)
        nc.sync.dma_start(out=bout_sb, in_=b_out)
        # ones column of vA via one strided copy (the v copies in phase 1
        # fill the rest).
        nc.vector.tensor_copy(
            out=vA[:, :, :, DH:DH + 1],
            in_=ones_sb[:, 0:R // 128 * HPC].rearrange(
                "p (a b c) -> p a b c", b=HPC, c=1))

        # ---------------- Phase 1: qkv projection ----------------
        with tc.tile_pool(name="p1", bufs=3) as p1, \
             tc.tile_pool(name="p1w", bufs=1) as p1w, \
             tc.tile_pool(name="ps1", bufs=2, space="PSUM") as ps1:
            wq = p1w.tile([128, NKD, 3 * HPC * DH], f32r)
            nc.sync.dma_start(out=wq,
                              in_=w_qkv.rearrange("(kt p) c -> p kt c", p=128))
            bq = p1w.tile([1, 3 * HPC * DH], bf16)
            nc.sync.dma_start(out=bq, in_=b_qkv)

            for rc in range(R // RC):
              with nc.named_scope(f"p1_c{rc}"):
                r0 = rc * RC
                xt = p1.tile([128, NKD, RC], f32r, tag="xt")
                nc.sync.dma_start(
                    out=xt,
                    in_=xT[:, r0:r0 + RC].rearrange("(kt p) r -> p kt r", p=128))
                ps_q = ps1.tile([128, RC], f32, tag="psq")
                ps_k = ps1.tile([128, RC], f32, tag="psk")
                ps_v = ps1.tile([128, RC], f32, tag="psv")
                for ps_t, c0 in ((ps_q, 0), (ps_k, 128), (ps_v, 256)):
                    nc.tensor.matmul(out=ps_t[:, :],
                                     lhsT=bq[0:1, c0:c0 + 128],
                                     rhs=ones_sb[0:1, 0:RC],
                                     start=True, stop=False)
                    for kt in range(NKD):
                        nc.tensor.matmul(out=ps_t[:, :],
                                         lhsT=wq[:, kt, c0:c0 + 128],
                                         rhs=xt[:, kt, :],
                                         start=False, stop=(kt == NKD - 1))
                nc.vector.tensor_copy(out=qT[:, r0:r0 + RC], in_=ps_q[:, :])
                nc.vector.tensor_copy(out=kT[:, r0:r0 + RC], in_=ps_k[:, :])
                nc.vector.tensor_copy(out=kT16[:, r0:r0 + RC], in_=ps_k[:, :])
                nc.sync.dma_start(out=kT_out[:, r0:r0 + RC],
                                  in_=kT[:, r0:r0 + RC])
                vt_sb = p1.tile([128, RC], f32, tag="vtsb")
                nc.vector.tensor_copy(out=vt_sb[:, :], in_=ps_v[:, :])
                nc.sync.dma_start(out=vT_out[:, r0:r0 + RC],
                                  in_=vt_sb[:, :])
                for t4 in range(RC // 128):
                    ps_vt = ps1.tile([128, 128], f32, tag="psvt")
                    nc.tensor.transpose(ps_vt[:, :],
                                        vt_sb[:, t4 * 128:(t4 + 1) * 128],
                                        ident)
                    rt = rc * (RC // 128) + t4
                    for hh in range(HPC):
                        nc.vector.tensor_copy(
                            out=vA[:, rt, hh, 0:DH],
                            in_=ps_vt[:, hh * DH:(hh + 1) * DH])

        # ---- Phase 2 (attention, heads packed as concurrent row-tiles) ----
        # ---- interleaved with per-batch A2A + Phase 3 out-proj ----
        with tc.tile_pool(name="p2", bufs=4) as p2, \
             tc.tile_pool(name="p2s", bufs=2) as p2s, \
             tc.tile_pool(name="p3", bufs=2) as p3, \
             tc.tile_pool(name="p3w", bufs=1) as p3w, \
             tc.tile_pool(name="p3o", bufs=4) as p3o, \
             tc.tile_pool(name="ps2s", bufs=2, space="PSUM") as ps2s, \
             tc.tile_pool(name="ps2c", bufs=1, space="PSUM") as ps2c, \
             tc.tile_pool(name="ps3", bufs=2, space="PSUM") as ps3:
            wo = p3w.tile([128, NKD, D], f32r)
            nc.sync.dma_start(out=wo,
                              in_=w_out.rearrange("(kt p) c -> p kt c", p=128))

            def attention(b):
              with nc.named_scope(f"att{b}"):
                for qc in range(NQC):
                    q0 = b * S + qc * QC
                    ctx = ps2c.tile([DH + 1, HPC, QC], f32, tag="ctx")
                    nk = (qc * QC) // 128 + 4
                    jbase = (qc * QC) // 128

                    def av(pend, last):
                        kt, off, pex = pend
                        for s in range(HPC):
                            nc.tensor.matmul(
                                out=ctx[:, s, off:QC],
                                lhsT=vA[:, (b * S) // 128 + kt, s, :],
                                rhs=pex[:, s, off:QC],
                                start=(kt == 0), stop=last)

                    pend = None
                    for kt in range(nk):
                        k0 = b * S + kt * 128
                        j = kt - jbase
                        off = 128 * j if j >= 0 else 0
                        scp = ps2s.tile([128, HPC, QC], f32, tag="sc")
                        # two half-array (K=64) score matmuls run concurrently
                        for s in range(HPC):
                            nc.tensor.matmul(
                                out=scp[:, s, :],
                                lhsT=kT16[DH * s:DH * s + DH, k0:k0 + 128],
                                rhs=qT[DH * s:DH * s + DH, q0:q0 + QC],
                                start=True, stop=True)
                        ex2 = p2.tile([128, HPC, QC], bf16, tag="ex")
                        nc.scalar.activation(out=ex2[:, :, off:QC],
                                             in_=scp[:, :, off:QC], func=EXP)
                        if j >= 0:
                            for s in range(HPC):
                                nc.vector.tensor_mul(
                                    ex2[:, s, off:QC], ex2[:, s, off:QC],
                                    triE[:, 384:384 + QC - off])
                        if pend is not None:
                            av(pend, False)
                        pend = (kt, off, ex2)
                    av(pend, True)
                    # normalize both heads: bcast sum, fast recip, multiply
                    for s in range(HPC):
                        se = p2s.tile([1, QC], f32, tag="se")
                        nc.vector.tensor_copy(out=se[:, :],
                                              in_=ctx[DH:DH + 1, s, :])
                        bc = p2s.tile([DH, QC], f32, tag="bc")
                        nc.gpsimd.partition_broadcast(bc[:, :], se[0:1, :])
                        nc.vector.reciprocal_approx_fast(out=bc[:, :],
                                                         in_=bc[:, :])
                        cx = p2s.tile([DH, QC], f32r, tag="cx")
                        nc.vector.tensor_mul(cx[:, :], ctx[0:DH, s, :],
                                             bc[:, :])
                        rel = qc * QC
                        for t in range(max(1, QC // W)):
                            j_sh = rel // W + t
                            ww = min(W, QC)
                            nc.sync.dma_start(
                                out=a2a_in[b][128 * j_sh + DH * s:
                                              128 * j_sh + DH * s + DH, :],
                                in_=cx[:, t * ww:(t + 1) * ww])

            def out_proj(b):
              with nc.named_scope(f"proj{b}"):
                ctxf = p3.tile([128, NKD, W], f32r, tag="ctxf")
                nc.sync.dma_start(
                    out=ctxf,
                    in_=a2a_out[b][:].rearrange("(kt p) r -> p kt r", p=128))
                for rt in range(W // 128):
                    for nch in range(D // 512):
                        ps_o = ps3.tile([128, 512], f32, tag="po")
                        nc.tensor.matmul(
                            out=ps_o[:, :],
                            lhsT=ones_sb[0:1, 0:128],
                            rhs=bout_sb[0:1, nch * 512:(nch + 1) * 512],
                            start=True, stop=False)
                        for kt in range(NKD):
                            nc.tensor.matmul(
                                out=ps_o[:, :],
                                lhsT=ctxf[:, kt, rt * 128:(rt + 1) * 128],
                                rhs=wo[:, kt, nch * 512:(nch + 1) * 512],
                                start=False, stop=(kt == NKD - 1))
                        ob = p3o.tile([128, 512], f32, tag="ob")
                        nc.vector.tensor_copy(out=ob[:, :], in_=ps_o[:, :])
                        nc.sync.dma_start(
                            out=o_out[b, rt * 128:(rt + 1) * 128,
                                      nch * 512:(nch + 1) * 512],
                            in_=ob[:, :])

            for b in range(B):
                attention(b)
                if b >= 2:
                    out_proj(b - 2)
                nc.gpsimd.collective_compute(
                    "AllToAll", mybir.AluOpType.bypass, replica_groups=RG,
                    ins=[a2a_in[b][:]], outs=[a2a_out[b][:]])
            out_proj(B - 2)
            out_proj(B - 1)

    nc.compile()
    return nc


def _get_nc():
    if "nc" not in _CACHE:
        _CACHE["nc"] = _build()
    return _CACHE["nc"]


def _host_inputs(x, w_in, b_in, w_out, b_out):
    """Build the 8 per-core input maps."""
    x = np.asarray(x, dtype=np.float32)
    w_in = np.asarray(w_in, dtype=np.float32)
    b_in = np.asarray(b_in, dtype=np.float32)
    w_out = np.asarray(w_out, dtype=np.float32)
    b_out = np.asarray(b_out, dtype=np.float32)
    Bb, Ss, _ = x.shape
    R = Bb * Ss

    xT = np.ascontiguousarray(x.reshape(R, D).T)
    scale = np.float32(1.0 / np.sqrt(DH))

    # causal triangle, extended for the 4 diagonal offsets:
    # triE[kk, u] = 1 iff u >= 384 + kk  (u in [0, 896))
    u = np.arange(896, dtype=np.int32)[None, :]
    kk = np.arange(128, dtype=np.int32)[:, None]
    triE = (u >= 384 + kk).astype(ml_dtypes.bfloat16)

    w_out_c = np.ascontiguousarray(w_out)
    b_out_c = b_out.reshape(1, D).astype(ml_dtypes.bfloat16)

    in_maps = []
    for c in range(NCORES):
        cols = slice(HPC * DH * c, HPC * DH * (c + 1))
        wq = w_in[:, 0:D][:, cols] * scale
        wk = w_in[:, D:2 * D][:, cols]
        wv = w_in[:, 2 * D:3 * D][:, cols]
        bq = b_in[0:D][cols] * scale
        bk = b_in[D:2 * D][cols]
        bv = b_in[2 * D:3 * D][cols]
        in_maps.append({
            "xT": xT,
            "w_qkv": np.ascontiguousarray(
                np.concatenate([wq, wk, wv], axis=1)),
            "b_qkv": np.concatenate([bq, bk, bv]).reshape(1, -1)
                .astype(ml_dtypes.bfloat16),
            "w_out": w_out_c,
            "b_out": b_out_c,
            "tri": triE,
            "cst": np.ones((128, 512), dtype=ml_dtypes.bfloat16),
        })
    return in_maps


def _assemble(results, B=4, S=2048):
    R = B * S
    W = S // NCORES
    k = np.empty((B, H, S, DH), dtype=np.float32)
    v = np.empty((B, H, S, DH), dtype=np.float32)
    out_flat = np.empty((R, D), dtype=np.float32)
    for c in range(NCORES):
        kc = results[c]["kT_out"].reshape(HPC, DH, B, S)
        vc = results[c]["vT_out"].reshape(HPC, DH, B, S)
        k[:, HPC * c:HPC * (c + 1)] = kc.transpose(2, 0, 3, 1)
        v[:, HPC * c:HPC * (c + 1)] = vc.transpose(2, 0, 3, 1)
        oc = results[c]["o_out"]          # [B, W, D]
        for b in range(B):
            g0 = b * S + W * c
            out_flat[g0:g0 + W] = oc[b]
    out = out_flat.reshape(B, S, D)
    return out, (k, v)


def kernel(x, w_in, b_in, w_out, b_out):
    from concourse.bass_utils import run_bass_kernel_spmd
    nc = _get_nc()
    in_maps = _host_inputs(x, w_in, b_in, w_out, b_out)
    res = run_bass_kernel_spmd(nc, in_maps, list(range(NCORES)))
    return _assemble(res.results)


# revision 18
# speedup vs baseline: 1.3910x; 1.2685x over previous
"""Multi-head causal attention (B=4, S=2048, D=1024, H=16) on 8 Trainium2 cores.

Strategy: tensor-parallel over heads (2 heads/core).
 - Host feeds each core xT = x^T [D, B*S] (fp32r + bf16 copies) plus that
   core's slice of w_in columns (q cols pre-scaled by 1/sqrt(dh)), and full
   w_out.
 - Phase 1: k,v chains in fp32r (output precision), q chain in bf16;
   qT/kT resident in SBUF; vT staged + PE-transposed to v-natural bf16 tiles
   with a ones column appended (flash-attention sum trick).
 - Phase 2 (bf16 matmuls): per (batch, head): scoresT[k,q] pairs of k-tiles
   into one 2-bank PSUM tile, one Exp per pair (ACT), causal mask multiply,
   ctxT[dh+1, q] accumulated on PE with v_aug stationary; row dh = sum(exp).
   Software-pipelined (next pair's scores before this pair's AV matmuls).
   Normalize via partition_broadcast + reciprocal_approx_fast.
 - Four AllToAlls (one per batch) reshard ctxT from head-split to row-split;
   all overlap attention/out-proj compute.
 - Phase 3: out rows-slice = ctxT_full^T @ w_out + b_out (fp32r).
Outputs per core: kT/vT head slices and out rows-slices; host reassembles.
"""

import numpy as np
import ml_dtypes
from contextlib import ExitStack

NCORES = 8
DH = 64
H = 16
HPC = H // NCORES          # heads per core = 2
D = H * DH                 # 1024
NKD = D // 128             # 8 contraction tiles over D
RC = 512                   # phase-1 row chunk
QC = 512                   # phase-2 query chunk

_CACHE = {}


def _build(B=4, S=2048):
    import concourse.tile as tile
    from concourse import bacc, mybir
    from concourse.masks import make_identity

    R = B * S
    W = S // NCORES        # per-batch A2A shard width (rows)
    NQC = S // QC
    f32 = mybir.dt.float32
    f32r = mybir.dt.float32r
    bf16 = mybir.dt.bfloat16
    EXP = mybir.ActivationFunctionType.Exp

    nc = bacc.Bacc("TRN2", target_bir_lowering=False, debug=False,
                   num_devices=NCORES)

    xT = nc.dram_tensor("xT", [D, R], f32r, kind="ExternalInput").ap()
    w_qkv = nc.dram_tensor("w_qkv", [D, 3 * HPC * DH], f32r,
                           kind="ExternalInput").ap()
    b_qkv = nc.dram_tensor("b_qkv", [1, 3 * HPC * DH], bf16,
                           kind="ExternalInput").ap()
    w_out = nc.dram_tensor("w_out", [D, D], bf16, kind="ExternalInput").ap()
    b_out = nc.dram_tensor("b_out", [1, D], bf16, kind="ExternalInput").ap()
    tri = nc.dram_tensor("tri", [128, 896], bf16, kind="ExternalInput").ap()
    cst = nc.dram_tensor("cst", [128, 512], bf16, kind="ExternalInput").ap()

    kT_out = nc.dram_tensor("kT_out", [HPC * DH, R], f32,
                            kind="ExternalOutput").ap()
    vT_out = nc.dram_tensor("vT_out", [HPC * DH, R], f32,
                            kind="ExternalOutput").ap()
    o_out = nc.dram_tensor("o_out", [B, W, D], f32,
                           kind="ExternalOutput").ap()

    # per-batch A2A buffers
    a2a_in = [nc.dram_tensor(f"a2a_in{b}", [NCORES * 128, W], bf16)
              for b in range(B)]
    a2a_out = [nc.dram_tensor(f"a2a_out{b}", [NCORES * 128, W], bf16)
               for b in range(B)]
    RG = [list(range(NCORES))]

    with tile.TileContext(nc) as tc, ExitStack() as ctx:
        persist = ctx.enter_context(tc.tile_pool(name="persist", bufs=1))
        qT = persist.tile([128, R], bf16)
        kT = persist.tile([128, R], f32)
        kT16 = persist.tile([128, R], bf16)
        vA = persist.tile([128, R // 128, HPC, DH + 1], bf16)
        triE = persist.tile([128, 896], bf16)
        ident = persist.tile([128, 128], f32)
        ones_sb = persist.tile([128, 512], bf16)
        bout_sb = persist.tile([1, D], bf16)

        make_identity(nc, ident)
        nc.sync.dma_start(out=ones_sb, in_=cst)
        nc.sync.dma_start(out=triE, in_=tri)
        nc.sync.dma_start(out=bout_sb, in_=b_out)
        # ones column of vA via one strided copy (the v copies in phase 1
        # fill the rest).
        nc.vector.tensor_copy(
            out=vA[:, :, :, DH:DH + 1],
            in_=ones_sb[:, 0:R // 128 * HPC].rearrange(
                "p (a b c) -> p a b c", b=HPC, c=1))

        # ---------------- Phase 1: qkv projection ----------------
        with tc.tile_pool(name="p1", bufs=3) as p1, \
             tc.tile_pool(name="p1w", bufs=1) as p1w, \
             tc.tile_pool(name="ps1", bufs=2, space="PSUM") as ps1:
            wq = p1w.tile([128, NKD, 3 * HPC * DH], f32r)
            nc.sync.dma_start(out=wq,
                              in_=w_qkv.rearrange("(kt p) c -> p kt c", p=128))
            bq = p1w.tile([1, 3 * HPC * DH], bf16)
            nc.sync.dma_start(out=bq, in_=b_qkv)

            for rc in range(R // RC):
              with nc.named_scope(f"p1_c{rc}"):
                r0 = rc * RC
                xt = p1.tile([128, NKD, RC], f32r, tag="xt")
                nc.sync.dma_start(
                    out=xt,
                    in_=xT[:, r0:r0 + RC].rearrange("(kt p) r -> p kt r", p=128))
                ps_q = ps1.tile([128, RC], f32, tag="psq")
                ps_k = ps1.tile([128, RC], f32, tag="psk")
                ps_v = ps1.tile([128, RC], f32, tag="psv")
                for ps_t, c0 in ((ps_q, 0), (ps_k, 128), (ps_v, 256)):
                    nc.tensor.matmul(out=ps_t[:, :],
                                     lhsT=bq[0:1, c0:c0 + 128],
                                     rhs=ones_sb[0:1, 0:RC],
                                     start=True, stop=False)
                    for kt in range(NKD):
                        nc.tensor.matmul(out=ps_t[:, :],
                                         lhsT=wq[:, kt, c0:c0 + 128],
                                         rhs=xt[:, kt, :],
                                         start=False, stop=(kt == NKD - 1))
                nc.vector.tensor_copy(out=qT[:, r0:r0 + RC], in_=ps_q[:, :])
                nc.vector.tensor_copy(out=kT[:, r0:r0 + RC], in_=ps_k[:, :])
                nc.vector.tensor_copy(out=kT16[:, r0:r0 + RC], in_=ps_k[:, :])
                nc.sync.dma_start(out=kT_out[:, r0:r0 + RC],
                                  in_=kT[:, r0:r0 + RC])
                vt_sb = p1.tile([128, RC], f32, tag="vtsb")
                nc.vector.tensor_copy(out=vt_sb[:, :], in_=ps_v[:, :])
                nc.sync.dma_start(out=vT_out[:, r0:r0 + RC],
                                  in_=vt_sb[:, :])
                for t4 in range(RC // 128):
                    ps_vt = ps1.tile([128, 128], f32, tag="psvt")
                    nc.tensor.transpose(ps_vt[:, :],
                                        vt_sb[:, t4 * 128:(t4 + 1) * 128],
                                        ident)
                    rt = rc * (RC // 128) + t4
                    for hh in range(HPC):
                        nc.vector.tensor_copy(
                            out=vA[:, rt, hh, 0:DH],
                            in_=ps_vt[:, hh * DH:(hh + 1) * DH])

        # ---- Phase 2 (attention, heads packed as concurrent row-tiles) ----
        # ---- interleaved with per-batch A2A + Phase 3 out-proj ----
        with tc.tile_pool(name="p2", bufs=4) as p2, \
             tc.tile_pool(name="p2s", bufs=2) as p2s, \
             tc.tile_pool(name="p3", bufs=2) as p3, \
             tc.tile_pool(name="p3w", bufs=1) as p3w, \
             tc.tile_pool(name="p3o", bufs=4) as p3o:
            wo = p3w.tile([128, NKD, D], bf16)
            nc.sync.dma_start(out=wo,
                              in_=w_out.rearrange("(kt p) c -> p kt c", p=128))

            def attention(b):
              with nc.named_scope(f"att{b}"):
                for qc in range(NQC):
                    q0 = b * S + qc * QC
                    ctx = ps2c.tile([DH + 1, HPC, QC], f32, tag="ctx")
                    nk = (qc * QC) // 128 + 4
                    jbase = (qc * QC) // 128

                    def av(pend, last):
                        kt, off, pex = pend
                        for s in range(HPC):
                            nc.tensor.matmul(
                                out=ctx[:, s, off:QC],
                                lhsT=vA[:, (b * S) // 128 + kt, s, :],
                                rhs=pex[:, s, off:QC],
                                start=(kt == 0), stop=last)

                    pend = None
                    for kt in range(nk):
                        k0 = b * S + kt * 128
                        j = kt - jbase
                        off = 128 * j if j >= 0 else 0
                        scp = ps2s.tile([128, HPC, QC], f32, tag="sc")
                        # two half-array (K=64) score matmuls run concurrently
                        for s in range(HPC):
                            nc.tensor.matmul(
                                out=scp[:, s, :],
                                lhsT=kT16[DH * s:DH * s + DH, k0:k0 + 128],
                                rhs=qT[DH * s:DH * s + DH, q0:q0 + QC],
                                start=True, stop=True)
                        ex2 = p2.tile([128, HPC, QC], bf16, tag="ex")
                        nc.scalar.activation(out=ex2[:, :, off:QC],
                                             in_=scp[:, :, off:QC], func=EXP)
                        if j >= 0:
                            for s in range(HPC):
                                nc.vector.tensor_mul(
                                    ex2[:, s, off:QC], ex2[:, s, off:QC],
                                    triE[:, 384:384 + QC - off])
                        if pend is not None:
                            av(pend, False)
                        pend = (kt, off, ex2)
                    av(pend, True)
                    # normalize both heads: bcast sum, fast recip, multiply
                    for s in range(HPC):
                        se = p2s.tile([1, QC], f32, tag="se")
                        nc.vector.tensor_copy(out=se[:, :],
                                              in_=ctx[DH:DH + 1, s, :])
                        bc = p2s.tile([DH, QC], f32, tag="bc")
                        nc.gpsimd.partition_broadcast(bc[:, :], se[0:1, :])
                        nc.vector.reciprocal_approx_fast(out=bc[:, :],
                                                         in_=bc[:, :])
                        cx = p2s.tile([DH, QC], bf16, tag="cx")
                        nc.vector.tensor_mul(cx[:, :], ctx[0:DH, s, :],
                                             bc[:, :])
                        rel = qc * QC
                        for t in range(max(1, QC // W)):
                            j_sh = rel // W + t
                            ww = min(W, QC)
                            nc.sync.dma_start(
                                out=a2a_in[b][128 * j_sh + DH * s:
                                              128 * j_sh + DH * s + DH, :],
                                in_=cx[:, t * ww:(t + 1) * ww])

            def out_proj(b, ps3):
              with nc.named_scope(f"proj{b}"):
                ctxf = p3.tile([128, NKD, W], bf16, tag="ctxf")
                nc.sync.dma_start(
                    out=ctxf,
                    in_=a2a_out[b][:].rearrange("(kt p) r -> p kt r", p=128))
                for rt in range(W // 128):
                    for nch in range(D // 512):
                        ps_o = ps3.tile([128, 512], f32, tag="po")
                        nc.tensor.matmul(
                            out=ps_o[:, :],
                            lhsT=ones_sb[0:1, 0:128],
                            rhs=bout_sb[0:1, nch * 512:(nch + 1) * 512],
                            start=True, stop=False)
                        for kt in range(NKD):
                            nc.tensor.matmul(
                                out=ps_o[:, :],
                                lhsT=ctxf[:, kt, rt * 128:(rt + 1) * 128],
                                rhs=wo[:, kt, nch * 512:(nch + 1) * 512],
                                start=False, stop=(kt == NKD - 1))
                        ob = p3o.tile([128, 512], f32, tag="ob")
                        nc.vector.tensor_copy(out=ob[:, :], in_=ps_o[:, :])
                        nc.sync.dma_start(
                            out=o_out[b, rt * 128:(rt + 1) * 128,
                                      nch * 512:(nch + 1) * 512],
                            in_=ob[:, :])

            with tc.tile_pool(name="ps2s", bufs=2, space="PSUM") as ps2s, \
                 tc.tile_pool(name="ps2c", bufs=2, space="PSUM") as ps2c:
                for b in range(B):
                    attention(b)
                    nc.gpsimd.collective_compute(
                        "AllToAll", mybir.AluOpType.bypass, replica_groups=RG,
                        ins=[a2a_in[b][:]], outs=[a2a_out[b][:]])
            with tc.tile_pool(name="ps3", bufs=2, space="PSUM") as ps3:
                for b in range(B):
                    out_proj(b, ps3)

    nc.compile()
    return nc


def _get_nc():
    if "nc" not in _CACHE:
        _CACHE["nc"] = _build()
    return _CACHE["nc"]


def _host_inputs(x, w_in, b_in, w_out, b_out):
    """Build the 8 per-core input maps."""
    x = np.asarray(x, dtype=np.float32)
    w_in = np.asarray(w_in, dtype=np.float32)
    b_in = np.asarray(b_in, dtype=np.float32)
    w_out = np.asarray(w_out, dtype=np.float32)
    b_out = np.asarray(b_out, dtype=np.float32)
    Bb, Ss, _ = x.shape
    R = Bb * Ss

    xT = np.ascontiguousarray(x.reshape(R, D).T)
    scale = np.float32(1.0 / np.sqrt(DH))

    # causal triangle, extended for the 4 diagonal offsets:
    # triE[kk, u] = 1 iff u >= 384 + kk  (u in [0, 896))
    u = np.arange(896, dtype=np.int32)[None, :]
    kk = np.arange(128, dtype=np.int32)[:, None]
    triE = (u >= 384 + kk).astype(ml_dtypes.bfloat16)

    w_out_c = np.ascontiguousarray(w_out).astype(ml_dtypes.bfloat16)
    b_out_c = b_out.reshape(1, D).astype(ml_dtypes.bfloat16)

    in_maps = []
    for c in range(NCORES):
        cols = slice(HPC * DH * c, HPC * DH * (c + 1))
        wq = w_in[:, 0:D][:, cols] * scale
        wk = w_in[:, D:2 * D][:, cols]
        wv = w_in[:, 2 * D:3 * D][:, cols]
        bq = b_in[0:D][cols] * scale
        bk = b_in[D:2 * D][cols]
        bv = b_in[2 * D:3 * D][cols]
        in_maps.append({
            "xT": xT,
            "w_qkv": np.ascontiguousarray(
                np.concatenate([wq, wk, wv], axis=1)),
            "b_qkv": np.concatenate([bq, bk, bv]).reshape(1, -1)
                .astype(ml_dtypes.bfloat16),
            "w_out": w_out_c,
            "b_out": b_out_c,
            "tri": triE,
            "cst": np.ones((128, 512), dtype=ml_dtypes.bfloat16),
        })
    return in_maps


def _assemble(results, B=4, S=2048):
    R = B * S
    W = S // NCORES
    k = np.empty((B, H, S, DH), dtype=np.float32)
    v = np.empty((B, H, S, DH), dtype=np.float32)
    out_flat = np.empty((R, D), dtype=np.float32)
    for c in range(NCORES):
        kc = results[c]["kT_out"].reshape(HPC, DH, B, S)
        vc = results[c]["vT_out"].reshape(HPC, DH, B, S)
        k[:, HPC * c:HPC * (c + 1)] = kc.transpose(2, 0, 3, 1)
        v[:, HPC * c:HPC * (c + 1)] = vc.transpose(2, 0, 3, 1)
        oc = results[c]["o_out"]          # [B, W, D]
        for b in range(B):
            g0 = b * S + W * c
            out_flat[g0:g0 + W] = oc[b]
    out = out_flat.reshape(B, S, D)
    return out, (k, v)


def kernel(x, w_in, b_in, w_out, b_out):
    from concourse.bass_utils import run_bass_kernel_spmd
    nc = _get_nc()
    in_maps = _host_inputs(x, w_in, b_in, w_out, b_out)
    res = run_bass_kernel_spmd(nc, in_maps, list(range(NCORES)))
    return _assemble(res.results)


# revision 20
# speedup vs baseline: 1.5234x; 1.0952x over previous
"""Multi-head causal attention (B=4, S=2048, D=1024, H=16) on 8 Trainium2 cores.

Strategy: tensor-parallel over heads (2 heads/core).
 - Host feeds each core xT = x^T [D, B*S] (fp32r + bf16 copies) plus that
   core's slice of w_in columns (q cols pre-scaled by 1/sqrt(dh)), and full
   w_out.
 - Phase 1: k,v chains in fp32r (output precision), q chain in bf16;
   qT/kT resident in SBUF; vT staged + PE-transposed to v-natural bf16 tiles
   with a ones column appended (flash-attention sum trick).
 - Phase 2 (bf16 matmuls): per (batch, head): scoresT[k,q] pairs of k-tiles
   into one 2-bank PSUM tile, one Exp per pair (ACT), causal mask multiply,
   ctxT[dh+1, q] accumulated on PE with v_aug stationary; row dh = sum(exp).
   Software-pipelined (next pair's scores before this pair's AV matmuls).
   Normalize via partition_broadcast + reciprocal_approx_fast.
 - Four AllToAlls (one per batch) reshard ctxT from head-split to row-split;
   all overlap attention/out-proj compute.
 - Phase 3: out rows-slice = ctxT_full^T @ w_out + b_out (fp32r).
Outputs per core: kT/vT head slices and out rows-slices; host reassembles.
"""

import numpy as np
import ml_dtypes
from contextlib import ExitStack

NCORES = 8
DH = 64
H = 16
HPC = H // NCORES          # heads per core = 2
D = H * DH                 # 1024
NKD = D // 128             # 8 contraction tiles over D
RC = 512                   # phase-1 row chunk
QC = 512                   # phase-2 query chunk

_CACHE = {}


def _build(B=4, S=2048):
    import concourse.tile as tile
    from concourse import bacc, mybir
    from concourse.masks import make_identity

    R = B * S
    W = S // NCORES        # per-batch A2A shard width (rows)
    NQC = S // QC
    f32 = mybir.dt.float32
    f32r = mybir.dt.float32r
    bf16 = mybir.dt.bfloat16
    EXP = mybir.ActivationFunctionType.Exp

    nc = bacc.Bacc("TRN2", target_bir_lowering=False, debug=False,
                   num_devices=NCORES)

    xT = nc.dram_tensor("xT", [D, R], f32r, kind="ExternalInput").ap()
    w_qkv = nc.dram_tensor("w_qkv", [D, 3 * HPC * DH], f32r,
                           kind="ExternalInput").ap()
    b_qkv = nc.dram_tensor("b_qkv", [1, 3 * HPC * DH], bf16,
                           kind="ExternalInput").ap()
    w_out = nc.dram_tensor("w_out", [D, D], bf16, kind="ExternalInput").ap()
    b_out = nc.dram_tensor("b_out", [1, D], bf16, kind="ExternalInput").ap()
    tri = nc.dram_tensor("tri", [128, 896], bf16, kind="ExternalInput").ap()
    cst = nc.dram_tensor("cst", [128, 512], bf16, kind="ExternalInput").ap()

    kT_out = nc.dram_tensor("kT_out", [HPC * DH, R], f32,
                            kind="ExternalOutput").ap()
    vT_out = nc.dram_tensor("vT_out", [HPC * DH, R], f32,
                            kind="ExternalOutput").ap()
    o_out = nc.dram_tensor("o_out", [B, W, D], f32,
                           kind="ExternalOutput").ap()

    # per-batch A2A buffers
    a2a_in = [nc.dram_tensor(f"a2a_in{b}", [NCORES * 128, W], bf16)
              for b in range(B)]
    a2a_out = [nc.dram_tensor(f"a2a_out{b}", [NCORES * 128, W], bf16)
               for b in range(B)]
    RG = [list(range(NCORES))]

    with tile.TileContext(nc) as tc, ExitStack() as ctx:
        persist = ctx.enter_context(tc.tile_pool(name="persist", bufs=1))
        qT = persist.tile([128, R], bf16)
        kT = persist.tile([128, R], f32)
        kT16 = persist.tile([128, R], bf16)
        vA = persist.tile([128, R // 128, HPC, 2 * DH], bf16)
        triE = persist.tile([128, 896], bf16)
        ident = persist.tile([128, 128], f32)
        ones_sb = persist.tile([128, 512], bf16)
        bout_sb = persist.tile([1, D], bf16)

        make_identity(nc, ident)
        nc.sync.dma_start(out=ones_sb, in_=cst)
        nc.sync.dma_start(out=triE, in_=tri)
        nc.sync.dma_start(out=bout_sb, in_=b_out)
        # fill vA with ones; phase-1 v copies overwrite cols [0, DH).
        # cols [DH, 2*DH) stay 1.0 so the AV matmul replicates sum(exp)
        # onto 64 psum partitions (normalizer broadcast for free).
        nc.vector.memset(vA[:, :, :, :], 1.0)

        # ---------------- Phase 1: qkv projection ----------------
        with tc.tile_pool(name="p1", bufs=3) as p1, \
             tc.tile_pool(name="p1w", bufs=1) as p1w, \
             tc.tile_pool(name="ps1", bufs=2, space="PSUM") as ps1:
            wq = p1w.tile([128, NKD, 3 * HPC * DH], f32r)
            nc.sync.dma_start(out=wq,
                              in_=w_qkv.rearrange("(kt p) c -> p kt c", p=128))
            bq = p1w.tile([1, 3 * HPC * DH], bf16)
            nc.sync.dma_start(out=bq, in_=b_qkv)

            for rc in range(R // RC):
              with nc.named_scope(f"p1_c{rc}"):
                r0 = rc * RC
                xt = p1.tile([128, NKD, RC], f32r, tag="xt")
                nc.sync.dma_start(
                    out=xt,
                    in_=xT[:, r0:r0 + RC].rearrange("(kt p) r -> p kt r", p=128))
                ps_q = ps1.tile([128, RC], f32, tag="psq")
                ps_k = ps1.tile([128, RC], f32, tag="psk")
                ps_v = ps1.tile([128, RC], f32, tag="psv")
                for ps_t, c0 in ((ps_q, 0), (ps_k, 128), (ps_v, 256)):
                    nc.tensor.matmul(out=ps_t[:, :],
                                     lhsT=bq[0:1, c0:c0 + 128],
                                     rhs=ones_sb[0:1, 0:RC],
                                     start=True, stop=False)
                    for kt in range(NKD):
                        nc.tensor.matmul(out=ps_t[:, :],
                                         lhsT=wq[:, kt, c0:c0 + 128],
                                         rhs=xt[:, kt, :],
                                         start=False, stop=(kt == NKD - 1))
                nc.vector.tensor_copy(out=qT[:, r0:r0 + RC], in_=ps_q[:, :])
                nc.vector.tensor_copy(out=kT[:, r0:r0 + RC], in_=ps_k[:, :])
                nc.vector.tensor_copy(out=kT16[:, r0:r0 + RC], in_=ps_k[:, :])
                nc.sync.dma_start(out=kT_out[:, r0:r0 + RC],
                                  in_=kT[:, r0:r0 + RC])
                vt_sb = p1.tile([128, RC], f32, tag="vtsb")
                nc.vector.tensor_copy(out=vt_sb[:, :], in_=ps_v[:, :])
                nc.sync.dma_start(out=vT_out[:, r0:r0 + RC],
                                  in_=vt_sb[:, :])
                for t4 in range(RC // 128):
                    ps_vt = ps1.tile([128, 128], f32, tag="psvt")
                    nc.tensor.transpose(ps_vt[:, :],
                                        vt_sb[:, t4 * 128:(t4 + 1) * 128],
                                        ident)
                    rt = rc * (RC // 128) + t4
                    for hh in range(HPC):
                        nc.vector.tensor_copy(
                            out=vA[:, rt, hh, 0:DH],
                            in_=ps_vt[:, hh * DH:(hh + 1) * DH])

        # ---- Phase 2 (attention, heads packed as concurrent row-tiles) ----
        # ---- interleaved with per-batch A2A + Phase 3 out-proj ----
        with tc.tile_pool(name="p2", bufs=4) as p2, \
             tc.tile_pool(name="p2s", bufs=2) as p2s, \
             tc.tile_pool(name="p3", bufs=2) as p3, \
             tc.tile_pool(name="p3w", bufs=1) as p3w, \
             tc.tile_pool(name="p3o", bufs=4) as p3o:
            wo = p3w.tile([128, NKD, D], bf16)
            nc.sync.dma_start(out=wo,
                              in_=w_out.rearrange("(kt p) c -> p kt c", p=128))

            def attention(b):
              with nc.named_scope(f"att{b}"):
                for qc in range(NQC):
                    q0 = b * S + qc * QC
                    ctx = ps2c.tile([2 * DH, HPC, QC], f32, tag="ctx")
                    nk = (qc * QC) // 128 + 4
                    jbase = (qc * QC) // 128

                    def av(pend, last):
                        kt, off, pex = pend
                        for s in range(HPC):
                            nc.tensor.matmul(
                                out=ctx[:, s, off:QC],
                                lhsT=vA[:, (b * S) // 128 + kt, s, :],
                                rhs=pex[:, s, off:QC],
                                start=(kt == 0), stop=last)

                    pend = None
                    for kt in range(nk):
                        k0 = b * S + kt * 128
                        j = kt - jbase
                        off = 128 * j if j >= 0 else 0
                        scp = ps2s.tile([128, HPC, QC], f32, tag="sc")
                        # two half-array (K=64) score matmuls run concurrently
                        for s in range(HPC):
                            nc.tensor.matmul(
                                out=scp[:, s, :],
                                lhsT=kT16[DH * s:DH * s + DH, k0:k0 + 128],
                                rhs=qT[DH * s:DH * s + DH, q0:q0 + QC],
                                start=True, stop=True)
                        ex2 = p2.tile([128, HPC, QC], bf16, tag="ex")
                        nc.scalar.activation(out=ex2[:, :, off:QC],
                                             in_=scp[:, :, off:QC], func=EXP)
                        if j >= 0:
                            for s in range(HPC):
                                nc.vector.tensor_mul(
                                    ex2[:, s, off:QC], ex2[:, s, off:QC],
                                    triE[:, 384:384 + QC - off])
                        if pend is not None:
                            av(pend, False)
                        pend = (kt, off, ex2)
                    av(pend, True)
                    # normalize both heads: bcast sum, fast recip, multiply
                    for s in range(HPC):
                        bc = p2s.tile([DH, QC], f32, tag="bc")
                        nc.vector.reciprocal_approx_fast(
                            out=bc[:, :], in_=ctx[DH:2 * DH, s, :])
                        cx = p2s.tile([DH, QC], bf16, tag="cx")
                        nc.vector.tensor_mul(cx[:, :], ctx[0:DH, s, :],
                                             bc[:, :])
                        rel = qc * QC
                        for t in range(max(1, QC // W)):
                            j_sh = rel // W + t
                            ww = min(W, QC)
                            nc.sync.dma_start(
                                out=a2a_in[b][128 * j_sh + DH * s:
                                              128 * j_sh + DH * s + DH, :],
                                in_=cx[:, t * ww:(t + 1) * ww])

            def out_proj(b, ps3):
              with nc.named_scope(f"proj{b}"):
                ctxf = p3.tile([128, NKD, W], bf16, tag="ctxf")
                nc.sync.dma_start(
                    out=ctxf,
                    in_=a2a_out[b][:].rearrange("(kt p) r -> p kt r", p=128))
                for rt in range(W // 128):
                    for nch in range(D // 512):
                        ps_o = ps3.tile([128, 512], f32, tag="po")
                        nc.tensor.matmul(
                            out=ps_o[:, :],
                            lhsT=ones_sb[0:1, 0:128],
                            rhs=bout_sb[0:1, nch * 512:(nch + 1) * 512],
                            start=True, stop=False)
                        for kt in range(NKD):
                            nc.tensor.matmul(
                                out=ps_o[:, :],
                                lhsT=ctxf[:, kt, rt * 128:(rt + 1) * 128],
                                rhs=wo[:, kt, nch * 512:(nch + 1) * 512],
                                start=False, stop=(kt == NKD - 1))
                        ob = p3o.tile([128, 512], f32, tag="ob")
                        nc.vector.tensor_copy(out=ob[:, :], in_=ps_o[:, :])
                        nc.sync.dma_start(
                            out=o_out[b, rt * 128:(rt + 1) * 128,
                                      nch * 512:(nch + 1) * 512],
                            in_=ob[:, :])

            with tc.tile_pool(name="ps2s", bufs=2, space="PSUM") as ps2s, \
                 tc.tile_pool(name="ps2c", bufs=2, space="PSUM") as ps2c:
                for b in range(B):
                    attention(b)
                    nc.gpsimd.collective_compute(
                        "AllToAll", mybir.AluOpType.bypass, replica_groups=RG,
                        ins=[a2a_in[b][:]], outs=[a2a_out[b][:]])
            with tc.tile_pool(name="ps3", bufs=2, space="PSUM") as ps3:
                for b in range(B):
                    out_proj(b, ps3)

    nc.compile()
    return nc


def _get_nc():
    if "nc" not in _CACHE:
        _CACHE["nc"] = _build()
    return _CACHE["nc"]


def _host_inputs(x, w_in, b_in, w_out, b_out):
    """Build the 8 per-core input maps."""
    x = np.asarray(x, dtype=np.float32)
    w_in = np.asarray(w_in, dtype=np.float32)
    b_in = np.asarray(b_in, dtype=np.float32)
    w_out = np.asarray(w_out, dtype=np.float32)
    b_out = np.asarray(b_out, dtype=np.float32)
    Bb, Ss, _ = x.shape
    R = Bb * Ss

    xT = np.ascontiguousarray(x.reshape(R, D).T)
    scale = np.float32(1.0 / np.sqrt(DH))

    # causal triangle, extended for the 4 diagonal offsets:
    # triE[kk, u] = 1 iff u >= 384 + kk  (u in [0, 896))
    u = np.arange(896, dtype=np.int32)[None, :]
    kk = np.arange(128, dtype=np.int32)[:, None]
    triE = (u >= 384 + kk).astype(ml_dtypes.bfloat16)

    w_out_c = np.ascontiguousarray(w_out).astype(ml_dtypes.bfloat16)
    b_out_c = b_out.reshape(1, D).astype(ml_dtypes.bfloat16)

    in_maps = []
    for c in range(NCORES):
        cols = slice(HPC * DH * c, HPC * DH * (c + 1))
        wq = w_in[:, 0:D][:, cols] * scale
        wk = w_in[:, D:2 * D][:, cols]
        wv = w_in[:, 2 * D:3 * D][:, cols]
        bq = b_in[0:D][cols] * scale
        bk = b_in[D:2 * D][cols]
        bv = b_in[2 * D:3 * D][cols]
        in_maps.append({
            "xT": xT,
            "w_qkv": np.ascontiguousarray(
                np.concatenate([wq, wk, wv], axis=1)),
            "b_qkv": np.concatenate([bq, bk, bv]).reshape(1, -1)
                .astype(ml_dtypes.bfloat16),
            "w_out": w_out_c,
            "b_out": b_out_c,
            "tri": triE,
            "cst": np.ones((128, 512), dtype=ml_dtypes.bfloat16),
        })
    return in_maps


def _assemble(results, B=4, S=2048):
    R = B * S
    W = S // NCORES
    k = np.empty((B, H, S, DH), dtype=np.float32)
    v = np.empty((B, H, S, DH), dtype=np.float32)
    out_flat = np.empty((R, D), dtype=np.float32)
    for c in range(NCORES):
        kc = results[c]["kT_out"].reshape(HPC, DH, B, S)
        vc = results[c]["vT_out"].reshape(HPC, DH, B, S)
        k[:, HPC * c:HPC * (c + 1)] = kc.transpose(2, 0, 3, 1)
        v[:, HPC * c:HPC * (c + 1)] = vc.transpose(2, 0, 3, 1)
        oc = results[c]["o_out"]          # [B, W, D]
        for b in range(B):
            g0 = b * S + W * c
            out_flat[g0:g0 + W] = oc[b]
    out = out_flat.reshape(B, S, D)
    return out, (k, v)


def kernel(x, w_in, b_in, w_out, b_out):
    from concourse.bass_utils import run_bass_kernel_spmd
    nc = _get_nc()
    in_maps = _host_inputs(x, w_in, b_in, w_out, b_out)
    res = run_bass_kernel_spmd(nc, in_maps, list(range(NCORES)))
    return _assemble(res.results)
